# revision 4
# baseline (speedup 1.0000x reference)
"""Trainium2 Bass kernel for BondEncoding2D (Graphormer-style bond encoding).

Computes, for a 512x512 node-pair grid:
  phi_spd[h,i,j]  = spd_table[spatial_pos[i,j], h]
  phi_edge[h,i,j] = (sum_d edge_table[edge_input[i,j,d]] @ W[d])[h] / max(spatial_pos[i,j],1)

Sharding: rows of the grid across 8 NeuronCores (64 rows / 32768 pairs each);
tables and weights replicated (per the sharding hint).

Per-core strategy (v2):
  * Host precomputes M[d] = edge_table @ W[d]; the edge term is then
    edge_sum[pair,:] = sum_d M[d, e_d(pair), :].
  * The (d,bond) one-hot over c=(d,b) [512 combos] and the spatial one-hot
    [64 values] are built ON HOST, stored as exact fp8 (0/1), and kept
    RESIDENT in SBUF (128 KB + 32 KB per partition).  The device loop is
    then just TensorEngine matmuls: the one-hot slice is the stationary
    operand (fp8 fast-weight-load), M (hi/lo fp8 split for ~bf16 accuracy)
    is the moving operand, accumulating the 16 hops in PSUM.
  * phi_spd: same pattern against the spatial one-hot, with the hi/lo halves
    merged directly in PSUM by accumulating matmul pairs.
  * Epilogues: DVE merges the edge hi/lo halves and applies 1/denom
    (broadcast multiply); ACT evacuates the spd PSUM.  Outputs staged as
    bf16 and upcast on host.
"""

import numpy as np
import ml_dtypes

import concourse.bass as bass
import concourse.bacc as bacc
import concourse.mybir as mybir
import concourse.tile as tile
from concourse.bass_utils import run_bass_kernel_spmd

N = 512          # atoms
D = 16           # max_dist
H = 32           # heads
NS = 64          # spatial values
NCORES = 8
RC = N // NCORES          # rows per core (64)
PC = RC * N               # pairs per core (32768)

TILES = 64                # tiles per core (one grid row each)
TP = 512                  # pairs per tile
NG = 4                    # groups of 128 pairs per tile
STAGE_T = 8               # tiles batched per output DMA

BF16 = mybir.dt.bfloat16
F32 = mybir.dt.float32
FP8 = mybir.dt.float8e4
NP8 = ml_dtypes.float8_e4m3

_cached = {}


def _build_nc(bench_reps=None, parts=("spd", "edge")):
    nc = bacc.Bacc(None, target_bir_lowering=False)

    ohe = nc.dram_tensor("ohe", [128, 4 * PC], FP8, kind="ExternalInput")
    ohs = nc.dram_tensor("ohs", [64, PC], FP8, kind="ExternalInput")
    mflat = nc.dram_tensor("mflat", [128, 256], FP8, kind="ExternalInput")
    stab = nc.dram_tensor("stab", [64, 64], FP8, kind="ExternalInput")
    rdev = nc.dram_tensor("rdev", [128, PC // 128], F32, kind="ExternalInput")
    oedge = nc.dram_tensor("oedge", [128, PC // 128 * H], BF16,
                           kind="ExternalOutput")
    ospd = nc.dram_tensor("ospd", [128, PC // 128 * H], BF16,
                          kind="ExternalOutput")

    add = mybir.AluOpType.add
    mult = mybir.AluOpType.mult

    with tile.TileContext(nc) as tc:
        with (
            tc.tile_pool(name="consts", bufs=1) as cpool,
            tc.tile_pool(name="op_e", bufs=2, space="PSUM") as oppool,
            tc.tile_pool(name="op_s", bufs=2, space="PSUM") as ospool,
            tc.tile_pool(name="tmp", bufs=2) as tmppool,
            tc.tile_pool(name="stage", bufs=2) as stpool,
            tc.tile_pool(name="stage_s", bufs=2) as sspool,
        ):
            ohe_t = cpool.tile([128, 4 * PC], FP8)
            nc.sync.dma_start(ohe_t[:], ohe[:])
            ohs_t = cpool.tile([64, PC], FP8)
            nc.sync.dma_start(ohs_t[:], ohs[:])
            mflat_t = cpool.tile([128, 256], FP8)
            nc.sync.dma_start(mflat_t[:], mflat[:])
            stab_t = cpool.tile([64, 64], FP8)
            nc.sync.dma_start(stab_t[:], stab[:])
            rdev_t = cpool.tile([128, PC // 128], F32)
            nc.sync.dma_start(rdev_t[:], rdev[:])

            import contextlib
            loop_cm = (
                tc.For_i(0, bench_reps, 1) if bench_reps
                else contextlib.nullcontext()
            )
            with loop_cm:
                for t in range(TILES):
                    sl = t % STAGE_T
                    if "edge" in parts:
                        op = oppool.tile([128, 256], F32, tag="op")
                        for gg in range(NG):
                            for q in range(4):
                                nc.tensor.matmul(
                                    op[:, 64 * gg:64 * gg + 64],
                                    ohe_t[:, q * PC + t * TP + gg * 128:
                                          q * PC + t * TP + gg * 128 + 128],
                                    mflat_t[:, 64 * q:64 * q + 64],
                                    start=(q == 0), stop=(q == 3),
                                )
                    if "spd" in parts:
                        ops = ospool.tile([128, 128], F32, tag="ops")
                        for gg in range(NG):
                            for hl in range(2):
                                nc.tensor.matmul(
                                    ops[:, 32 * gg:32 * gg + 32],
                                    ohs_t[:, t * TP + gg * 128:
                                          t * TP + gg * 128 + 128],
                                    stab_t[:, 32 * hl:32 * hl + 32],
                                    start=(hl == 0), stop=(hl == 1),
                                )
                    if "edge" in parts:
                        # (hi + lo) * (1/denom) -> staging
                        if sl == 0:
                            st = stpool.tile([128, 128 * STAGE_T], BF16,
                                             tag="st")
                        opv = op[:].rearrange("p (g k s) -> p g k s",
                                              g=NG, k=H)
                        tmp = tmppool.tile([128, 128], F32, tag="tmp")
                        tmpv = tmp[:].rearrange("p (g k) -> p g k", g=NG)
                        nc.vector.reduce_sum(
                            tmpv, opv, axis=mybir.AxisListType.X)
                        r4 = rdev_t[:, t * NG:(t + 1) * NG]
                        r4b = r4.rearrange("p (g o) -> p g o", o=1)
                        r4b = r4b.broadcast_to((128, NG, H))
                        stv = st[:, 128 * sl:128 * (sl + 1)].rearrange(
                            "p (g k) -> p g k", g=NG)
                        nc.vector.tensor_tensor(stv, tmpv, r4b, mult)
                        if sl == STAGE_T - 1:
                            b = t // STAGE_T
                            nc.sync.dma_start(
                                oedge[:, b * 128 * STAGE_T:
                                      (b + 1) * 128 * STAGE_T],
                                st[:],
                            )
                    if "spd" in parts:
                        if sl == 0:
                            sts = sspool.tile([128, 128 * STAGE_T], BF16,
                                              tag="sts")
                        nc.scalar.copy(
                            sts[:, 128 * sl:128 * (sl + 1)], ops[:])
                        if sl == STAGE_T - 1:
                            b = t // STAGE_T
                            nc.sync.dma_start(
                                ospd[:, b * 128 * STAGE_T:
                                     (b + 1) * 128 * STAGE_T],
                                sts[:],
                            )
    nc.compile()
    return nc


def _host_prep(spatial_pos, edge_input, max_dist, spd_table, edge_table,
               edge_dis_weight):
    """Build per-core input maps (all numpy)."""
    md = int(max_dist)
    assert md == D
    W = edge_dis_weight.reshape(-1, H, H)[:md].astype(np.float64)
    M = edge_table.astype(np.float64) @ W          # (16, 32, 32)

    cp = np.arange(128)
    # mflat[c', 64q + (hi|lo)*32 + k] : fp8 hi/lo split of M[4q+c'//32, c'%32, k]
    mflat = np.zeros((128, 256), NP8)
    for q in range(4):
        blk = M[4 * q + cp // 32, cp % 32, :]      # (128, 32) float64
        hi = blk.astype(NP8)
        lo = (blk - hi.astype(np.float64)).astype(NP8)
        mflat[:, 64 * q:64 * q + 64:2] = hi        # interleave (k, hi|lo)
        mflat[:, 64 * q + 1:64 * q + 64:2] = lo

    st64 = spd_table.astype(np.float64)            # (64, 32)
    shi = st64.astype(NP8)
    slo = (st64 - shi.astype(np.float64)).astype(NP8)
    stab = np.concatenate([shi, slo], axis=1)      # (64, 64)

    bonds = np.arange(32, dtype=np.int32)
    svals = np.arange(64, dtype=np.int32)
    in_maps = []
    for c in range(NCORES):
        rows = slice(RC * c, RC * (c + 1))
        e = edge_input[rows].reshape(PC, D)
        ohe = np.zeros((128, 4 * PC), NP8)
        for q in range(4):
            eq = e[:, 4 * q:4 * q + 4]             # (PC, 4)
            oh = (eq[:, :, None] == bonds[None, None, :])  # (PC, 4, 32)
            ohe[:, q * PC:(q + 1) * PC] = oh.reshape(PC, 128).T
        sp = spatial_pos[rows].reshape(PC)
        ohs = np.ascontiguousarray(
            (sp[None, :] == svals[:, None])).astype(NP8)   # (64, PC)
        rdevv = (1.0 / np.maximum(sp, 1)).astype(np.float32)
        rdevv = np.ascontiguousarray(rdevv.reshape(PC // 128, 128).T)
        in_maps.append({
            "ohe": ohe, "ohs": ohs, "mflat": mflat, "stab": stab,
            "rdev": rdevv,
        })
    return in_maps


def _host_assemble(results):
    phi_spd = np.empty((H, N, N), np.float32)
    phi_edge = np.empty((H, N, N), np.float32)
    for c in range(NCORES):
        rs = slice(RC * c, RC * (c + 1))
        a = np.asarray(results[c]["ospd"]).astype(np.float32)
        a = a.reshape(128, TILES, NG, H)
        phi_spd[:, rs, :] = a.transpose(3, 1, 2, 0).reshape(H, RC, N)
        b = np.asarray(results[c]["oedge"]).astype(np.float32)
        b = b.reshape(128, TILES, NG, H)
        phi_edge[:, rs, :] = b.transpose(3, 1, 2, 0).reshape(H, RC, N)
    return phi_spd, phi_edge


def kernel(spatial_pos, edge_input, max_dist, spd_table, edge_table,
           edge_dis_weight, _trace=False):
    spatial_pos = np.asarray(spatial_pos)
    edge_input = np.asarray(edge_input)
    spd_table = np.asarray(spd_table, dtype=np.float32)
    edge_table = np.asarray(edge_table, dtype=np.float32)
    edge_dis_weight = np.asarray(edge_dis_weight, dtype=np.float32)

    if "nc" not in _cached:
        _cached["nc"] = _build_nc()
    nc = _cached["nc"]

    in_maps = _host_prep(spatial_pos, edge_input, max_dist, spd_table,
                         edge_table, edge_dis_weight)
    res = run_bass_kernel_spmd(
        nc, in_maps, core_ids=list(range(NCORES)), trace=bool(_trace)
    )
    out = _host_assemble(res.results)
    if _trace:
        return out, res
    return out


# revision 7
# speedup vs baseline: 2.4135x; 2.4135x over previous
"""Trainium2 Bass kernel for BondEncoding2D (Graphormer-style bond encoding).

Computes, for a 512x512 node-pair grid:
  phi_spd[h,i,j]  = spd_table[spatial_pos[i,j], h]
  phi_edge[h,i,j] = (sum_d edge_table[edge_input[i,j,d]] @ W[d])[h] / max(spatial_pos[i,j],1)

Sharding: rows of the grid across 8 NeuronCores (64 rows / 32768 pairs each);
tables and weights replicated (per the sharding hint).

Per-core strategy (v2):
  * Host precomputes M[d] = edge_table @ W[d]; the edge term is then
    edge_sum[pair,:] = sum_d M[d, e_d(pair), :].
  * The (d,bond) one-hot over c=(d,b) [512 combos] and the spatial one-hot
    [64 values] are built ON HOST, stored as exact fp8 (0/1), and kept
    RESIDENT in SBUF (128 KB + 32 KB per partition).  The device loop is
    then just TensorEngine matmuls: the one-hot slice is the stationary
    operand (fp8 fast-weight-load), M (hi/lo fp8 split for ~bf16 accuracy)
    is the moving operand, accumulating the 16 hops x {hi,lo} in PSUM.
  * phi_spd: same pattern against the spatial one-hot.
  * Epilogues: DVE applies 1/denom (broadcast multiply) to the edge PSUM;
    ACT evacuates the spd PSUM.  Outputs staged as bf16, upcast on host.
"""

import numpy as np
import ml_dtypes

import concourse.bass as bass
import concourse.bacc as bacc
import concourse.mybir as mybir
import concourse.tile as tile
from concourse.bass_utils import run_bass_kernel_spmd

N = 512          # atoms
D = 16           # max_dist
H = 32           # heads
NS = 64          # spatial values
NCORES = 8
RC = N // NCORES          # rows per core (64)
PC = RC * N               # pairs per core (32768)

TILES = 64                # tiles per core (one grid row each)
TP = 512                  # pairs per tile
NG = 4                    # groups of 128 pairs per tile
STAGE_T = 8               # tiles batched per output DMA

BF16 = mybir.dt.bfloat16
F32 = mybir.dt.float32
FP8 = mybir.dt.float8e4
NP8 = ml_dtypes.float8_e4m3

_cached = {}


def _build_nc(bench_reps=None, parts=("spd", "edge")):
    # expand coarse part names into component flags
    flags = set()
    for p in parts:
        if p in ("spd", "edge"):
            flags |= {p + "_mm", p + "_epi", p + "_dma"}
        else:
            flags.add(p)
    nc = bacc.Bacc(None, target_bir_lowering=False)

    ohe = nc.dram_tensor("ohe", [128, 4 * PC], FP8, kind="ExternalInput")
    ohs = nc.dram_tensor("ohs", [64, PC], FP8, kind="ExternalInput")
    mflat = nc.dram_tensor("mflat", [128, 256], FP8, kind="ExternalInput")
    stab = nc.dram_tensor("stab", [64, 64], FP8, kind="ExternalInput")
    rdev = nc.dram_tensor("rdev", [128, PC // 128], F32, kind="ExternalInput")
    oedge = nc.dram_tensor("oedge", [128, PC // 128 * H], BF16,
                           kind="ExternalOutput")
    ospd = nc.dram_tensor("ospd", [128, PC // 128 * H], BF16,
                          kind="ExternalOutput")

    mult = mybir.AluOpType.mult

    with tile.TileContext(nc) as tc:
        with (
            tc.tile_pool(name="consts", bufs=1) as cpool,
            tc.tile_pool(name="op_e", bufs=2, space="PSUM") as oppool,
            tc.tile_pool(name="op_s", bufs=2, space="PSUM") as ospool,
            tc.tile_pool(name="stage", bufs=2) as stpool,
            tc.tile_pool(name="stage_s", bufs=2) as sspool,
        ):
            ohe_t = cpool.tile([128, 4 * PC], FP8)
            nc.sync.dma_start(ohe_t[:], ohe[:])
            ohs_t = cpool.tile([64, PC], FP8)
            nc.sync.dma_start(ohs_t[:], ohs[:])
            mflat_t = cpool.tile([128, 256], FP8)
            nc.sync.dma_start(mflat_t[:], mflat[:])
            stab_t = cpool.tile([64, 64], FP8)
            nc.sync.dma_start(stab_t[:], stab[:])
            rdev_t = cpool.tile([128, PC // 128], F32)
            nc.sync.dma_start(rdev_t[:], rdev[:])

            import contextlib
            loop_cm = (
                tc.For_i(0, bench_reps, 1) if bench_reps
                else contextlib.nullcontext()
            )
            with loop_cm:
                for t in range(TILES):
                    sl = t % STAGE_T
                    if "edge_mm" in flags:
                        # full-bank tile: only cols 0:128 used
                        op = oppool.tile([128, 512], F32, tag="op")
                        for gg in range(NG):
                            for j in range(8):
                                q, hl = j // 2, j % 2
                                nc.tensor.matmul(
                                    op[:, 32 * gg:32 * gg + 32],
                                    ohe_t[:, t * 2048 + q * 512 + gg * 128:
                                          t * 2048 + q * 512 + gg * 128 + 128],
                                    mflat_t[:, 64 * q + 32 * hl:
                                            64 * q + 32 * hl + 32],
                                    start=(j == 0), stop=(j == 7),
                                )
                    if "spd_mm" in flags:
                        ops = ospool.tile([128, 512], F32, tag="ops")
                        for gg in range(NG):
                            for hl in range(2):
                                nc.tensor.matmul(
                                    ops[:, 32 * gg:32 * gg + 32],
                                    ohs_t[:, t * TP + gg * 128:
                                          t * TP + gg * 128 + 128],
                                    stab_t[:, 32 * hl:32 * hl + 32],
                                    start=(hl == 0), stop=(hl == 1),
                                )
                    if "edge_epi" in flags or "edge_dma" in flags:
                        if sl == 0:
                            st = stpool.tile([128, 128 * STAGE_T], BF16,
                                             tag="st")
                    if "edge_epi" in flags:
                        # x (1/denom) -> staging
                        opv = op[:, 0:128].rearrange("p (g k) -> p g k", g=NG)
                        r4 = rdev_t[:, t * NG:(t + 1) * NG]
                        r4b = r4.rearrange("p (g o) -> p g o", o=1)
                        r4b = r4b.broadcast_to((128, NG, H))
                        stv = st[:, 128 * sl:128 * (sl + 1)].rearrange(
                            "p (g k) -> p g k", g=NG)
                        nc.vector.tensor_tensor(stv, opv, r4b, mult)
                    if "edge_dma" in flags:
                        if sl == STAGE_T - 1:
                            b = t // STAGE_T
                            nc.sync.dma_start(
                                oedge[:, b * 128 * STAGE_T:
                                      (b + 1) * 128 * STAGE_T],
                                st[:],
                            )
                    if "spd_epi" in flags or "spd_dma" in flags:
                        if sl == 0:
                            sts = sspool.tile([128, 128 * STAGE_T], BF16,
                                              tag="sts")
                    if "spd_epi" in flags:
                        nc.scalar.copy(
                            sts[:, 128 * sl:128 * (sl + 1)], ops[:, 0:128])
                    if "spd_dma" in flags:
                        if sl == STAGE_T - 1:
                            b = t // STAGE_T
                            nc.sync.dma_start(
                                ospd[:, b * 128 * STAGE_T:
                                     (b + 1) * 128 * STAGE_T],
                                sts[:],
                            )
    nc.compile()
    return nc


def _host_prep(spatial_pos, edge_input, max_dist, spd_table, edge_table,
               edge_dis_weight):
    """Build per-core input maps (all numpy)."""
    md = int(max_dist)
    assert md == D
    W = edge_dis_weight.reshape(-1, H, H)[:md].astype(np.float64)
    M = edge_table.astype(np.float64) @ W          # (16, 32, 32)

    cp = np.arange(128)
    # mflat[c', 64q + 32*(hi|lo) + k] : fp8 hi/lo split of M[4q+c'//32, c'%32, k]
    mflat = np.zeros((128, 256), NP8)
    for q in range(4):
        blk = M[4 * q + cp // 32, cp % 32, :]      # (128, 32) float64
        hi = blk.astype(NP8)
        lo = (blk - hi.astype(np.float64)).astype(NP8)
        mflat[:, 64 * q:64 * q + 32] = hi
        mflat[:, 64 * q + 32:64 * q + 64] = lo

    st64 = spd_table.astype(np.float64)            # (64, 32)
    shi = st64.astype(NP8)
    slo = (st64 - shi.astype(np.float64)).astype(NP8)
    stab = np.concatenate([shi, slo], axis=1)      # (64, 64)

    bonds = np.arange(32, dtype=np.int32)
    svals = np.arange(64, dtype=np.int32)
    in_maps = []
    for c in range(NCORES):
        rows = slice(RC * c, RC * (c + 1))
        e = edge_input[rows].reshape(PC, D)
        # tile-contiguous layout: [128, (t, q, gg*128)]
        # oh4[r, t, q, j] = (e[t*512+j, 4q + r//32] == r%32)
        eq = e.reshape(TILES, TP, 4, 4)            # (t, j, q, dd)
        oh = (eq[:, :, :, :, None] == bonds[None, None, None, None, :])
        # -> (128=dd*32+b, t, q, j)
        oh = oh.transpose(3, 4, 0, 2, 1)           # (dd, b, t, q, j)
        ohe = np.ascontiguousarray(
            oh.reshape(128, TILES, 4, TP).reshape(128, 4 * PC)).astype(NP8)
        sp = spatial_pos[rows].reshape(PC)
        ohs = np.ascontiguousarray(
            (sp[None, :] == svals[:, None])).astype(NP8)   # (64, PC)
        rdevv = (1.0 / np.maximum(sp, 1)).astype(np.float32)
        rdevv = np.ascontiguousarray(rdevv.reshape(PC // 128, 128).T)
        in_maps.append({
            "ohe": ohe, "ohs": ohs, "mflat": mflat, "stab": stab,
            "rdev": rdevv,
        })
    return in_maps


def _host_assemble(results):
    phi_spd = np.empty((H, N, N), np.float32)
    phi_edge = np.empty((H, N, N), np.float32)
    for c in range(NCORES):
        rs = slice(RC * c, RC * (c + 1))
        a = np.asarray(results[c]["ospd"]).astype(np.float32)
        a = a.reshape(128, TILES, NG, H)
        phi_spd[:, rs, :] = a.transpose(3, 1, 2, 0).reshape(H, RC, N)
        b = np.asarray(results[c]["oedge"]).astype(np.float32)
        b = b.reshape(128, TILES, NG, H)
        phi_edge[:, rs, :] = b.transpose(3, 1, 2, 0).reshape(H, RC, N)
    return phi_spd, phi_edge


def kernel(spatial_pos, edge_input, max_dist, spd_table, edge_table,
           edge_dis_weight, _trace=False):
    spatial_pos = np.asarray(spatial_pos)
    edge_input = np.asarray(edge_input)
    spd_table = np.asarray(spd_table, dtype=np.float32)
    edge_table = np.asarray(edge_table, dtype=np.float32)
    edge_dis_weight = np.asarray(edge_dis_weight, dtype=np.float32)

    if "nc" not in _cached:
        _cached["nc"] = _build_nc()
    nc = _cached["nc"]

    in_maps = _host_prep(spatial_pos, edge_input, max_dist, spd_table,
                         edge_table, edge_dis_weight)
    res = run_bass_kernel_spmd(
        nc, in_maps, core_ids=list(range(NCORES)), trace=bool(_trace)
    )
    out = _host_assemble(res.results)
    if _trace:
        return out, res
    return out
